# revision 27
# baseline (speedup 1.0000x reference)
"""Cross-image contrastive loss on 8 TRN2 NeuronCores.

Strategy (row-parallel over N=4096 pixels, 512 rows/core, rows sorted by label):
  - S1 (sum over exp(Fi.Fii/tau)): full [512, 4096] pass, fp8e4m3 matmuls
    (x8-scaled features; psum = 64*logits, folded into the exp scale).
  - S2 (label-masked sum): rows AND columns sorted by label; each 128-row
    block only needs the jj-columns matching its (at most 2) labels: a
    2x320-column window instead of 4096. The mask is applied by the DVE as
    a per-partition 0/1 multiply fused into the masked row-sum
    (tensor_scalar accum_out); zero-padded columns contribute exp(0)=1 and
    are subtracted exactly via a host-side per-row offset that rides in the
    accumulator alongside the +EPS term.
  - exp work is split between the Scalar engine (activation Exp, fused
    accum row-sum) and the Vector engine (Schraudolph exponential:
    y = A*psum + B computed into int32, bitcast to f32, summed; the
    bias constant is calibrated so the approximation is mean-zero, and it
    covers only 1/4 of each affected row's Z, keeping the loss error
    ~1e-4).
  - Histograms, per-pixel weights w, and the diagonal terms are O(N),
    computed on host; the device does all O(N^2) work.
  - Each core emits sum(2*w*logZ)/N; host subtracts sum(w*diag)/N and sums
    the 8 partials.
"""

import math
import sys

import numpy as np

sys.path.insert(0, "/opt/trn_rl_repo")

import ml_dtypes

TAU = 0.07
EPS = 1e-4
L = 19
D = 64
N = 4096
NCORES = 8
P = N // NCORES  # 512 rows per core
PB = P // 128  # 4 partition blocks per core
SW = 320  # S2 segment width (max label count in data ~238)
FSCL = 8.0  # fp8 feature scale; psum = FSCL^2 * logits
MSC = FSCL * FSCL
ESC = 1.0 / (MSC * TAU)  # exp scale applied to psum
# Schraudolph: exp(u) ~ bitcast_f32(int32(A*psum + B)); C calibrated mean-zero
SCH_C = 0.0562
SCH_A = (1 << 23) * ESC / math.log(2.0)
SCH_B = (1 << 23) * (127.0 - SCH_C) + 0.5

_compiled = None
_LDW_PATCHED = False


def _enable_ldw_opt():
    """Flip walrus --enable-ldw-opt for this process (dedups back-to-back
    LDWEIGHTS of the same stationary operand)."""
    global _LDW_PATCHED
    if _LDW_PATCHED:
        return
    from concourse import bass_utils

    orig = bass_utils.run_command

    def patched(cmd, *a, **kw):
        if isinstance(cmd, list):
            cmd = [
                "--enable-ldw-opt=true" if c == "--enable-ldw-opt=false" else c
                for c in cmd
            ]
        return orig(cmd, *a, **kw)

    bass_utils.run_command = patched
    _LDW_PATCHED = True


def _build():
    from concourse import bacc, mybir, tile

    f32 = mybir.dt.float32
    i32 = mybir.dt.int32
    bf16 = mybir.dt.bfloat16
    f8 = mybir.dt.float8e4
    Exp = mybir.ActivationFunctionType.Exp
    Ln = mybir.ActivationFunctionType.Ln
    X = mybir.AxisListType.X
    add = mybir.AluOpType.add
    mult = mybir.AluOpType.mult

    nc = bacc.Bacc("TRN2", target_bir_lowering=False, debug=False)

    lhsP_d = nc.dram_tensor("lhsP", (128, 2 * 128), f8, kind="ExternalInput")
    rhs1_d = nc.dram_tensor("rhs1", (128, N), f8, kind="ExternalInput")
    rhs2_d = nc.dram_tensor("rhs2", (128, 2 * 2 * SW), f8, kind="ExternalInput")
    small_d = nc.dram_tensor("small", (128, 17), f32, kind="ExternalInput")
    out_d = nc.dram_tensor("out", (1, 1), f32, kind="ExternalOutput")

    with tile.TileContext(nc) as tc:
        with (
            tc.tile_pool(name="res", bufs=1) as res,
            tc.tile_pool(name="scr", bufs=3) as scr,
            tc.tile_pool(name="yi", bufs=2) as yip,
            tc.tile_pool(name="ps", bufs=4, space="PSUM") as psp,
        ):
            lhsP_sb = res.tile([128, 2 * 128], f8, tag="lhsP")
            rhs1_sb = res.tile([128, N], f8, tag="rhs1")
            rhs2_sb = res.tile([128, 2 * 2 * SW], f8, tag="rhs2")
            small_sb = res.tile([128, 17], f32, tag="small")
            acc = res.tile([128, 28], f32, tag="acc")  # col = j*4 + t
            zeros = res.tile([128, 1], f32, tag="zeros")
            nc.vector.memset(zeros[:], 0.0)

            # DMA order: critical path (lhsP + first rhs1 pieces) first; the
            # leading 512-col pieces shrink time-to-first-matmul
            nc.sync.dma_start(rhs1_sb[:, 0:512], rhs1_d[:, 0:512])
            nc.sync.dma_start(lhsP_sb[:], lhsP_d[:])
            for c0, c1 in ((512, 1024), (1024, 2048), (2048, 3072), (3072, 4096)):
                nc.sync.dma_start(rhs1_sb[:, c0:c1], rhs1_d[:, c0:c1])
            nc.scalar.dma_start(rhs2_sb[:], rhs2_d[:])
            nc.scalar.dma_start(small_sb[:], small_d[:])
            nc.sync.dma_start(acc[:, 24:28], small_d[:, 8:12])  # zoff as j=6

            nc.scalar.add_instruction(
                mybir.InstLoadActFuncSet(
                    name=nc.get_next_instruction_name(),
                    act_func_set_id=6,  # natural_log_exp_and_others
                    ins=[],
                    outs=[],
                )
            )

            mask = small_sb[:, 0:8]  # col t*2+s
            w2 = small_sb[:, 12:16]
            ones = small_sb[:, 16:17]

            def s1_unit(t, c, wsl, rsl, tp, dve):
                ps = psp.tile([128, 1024], f32, tag="mm")
                for m in range(2):
                    nc.tensor.matmul(
                        ps[:, m * 512 : (m + 1) * 512],
                        lhsP_sb[wsl[0] : wsl[1], wsl[2] : wsl[3]],
                        rhs1_sb[rsl[0] : rsl[1], c * 1024 + m * 512 : c * 1024 + (m + 1) * 512],
                        start=True,
                        stop=True,
                        tile_position=tp,
                    )
                col = c * 4 + t
                if dve:
                    yi = yip.tile([128, 1024], i32, tag="yi")
                    nc.vector.tensor_scalar(yi[:], ps[:], SCH_A, SCH_B, mult, add)
                    nc.vector.tensor_reduce(
                        acc[:, col : col + 1], yi[:].bitcast(f32), axis=X, op=add
                    )
                else:
                    dump = scr.tile([128, 1024], bf16, tag="dump")
                    nc.scalar.activation(
                        dump[:],
                        ps[:],
                        Exp,
                        bias=zeros[:],
                        scale=ESC,
                        accum_out=acc[:, col : col + 1],
                    )

            def s2_unit(t, q, wsl, rsl, tp):
                ps = psp.tile([128, 1024], f32, tag="mm")
                w0 = q * 2 * SW
                for c0, c1 in ((0, 512), (512, 2 * SW)):
                    nc.tensor.matmul(
                        ps[:, c0:c1],
                        lhsP_sb[wsl[0] : wsl[1], wsl[2] : wsl[3]],
                        rhs2_sb[rsl[0] : rsl[1], w0 + c0 : w0 + c1],
                        start=True,
                        stop=True,
                        tile_position=tp,
                    )
                dump = scr.tile([128, 2 * SW], bf16, tag="dump2")
                nc.scalar.activation(
                    dump[:], ps[:, 0 : 2 * SW], Exp, bias=zeros[:], scale=ESC
                )
                for s in range(2):
                    tmp = scr.tile([128, SW], bf16, tag="tmp2")
                    nc.vector.tensor_scalar(
                        tmp[:],
                        dump[:, s * SW : (s + 1) * SW],
                        mask[:, t * 2 + s : t * 2 + s + 1],
                        None,
                        mult,
                        add,
                        accum_out=acc[:, (4 + s) * 4 + t : (4 + s) * 4 + t + 1],
                    )

            for q in range(2):
                wA = (0, 64, q * 128, (q + 1) * 128)
                wB = (64, 128, q * 128, (q + 1) * 128)
                rA, rB = (0, 64), (64, 128)
                tA, tB = 2 * q, 2 * q + 1
                s1_unit(tA, 0, wA, rA, (0, 0), dve=False)
                s2_unit(tA, q, wA, rA, (0, 0))
                s1_unit(tA, 1, wA, rA, (0, 0), dve=True)
                s1_unit(tA, 2, wA, rA, (0, 0), dve=False)
                s1_unit(tA, 3, wA, rA, (0, 0), dve=False)
                s1_unit(tB, 0, wB, rB, (64, 0), dve=False)
                s2_unit(tB, q, wB, rB, (64, 0))
                s1_unit(tB, 1, wB, rB, (64, 0), dve=True)
                s1_unit(tB, 2, wB, rB, (64, 0), dve=(q == 0))
                s1_unit(tB, 3, wB, rB, (64, 0), dve=False)

            # ---- Z = sum_j acc[j, t] (incl. zoff), logZ, partial ----
            z = res.tile([128, PB], f32, tag="z")
            nc.vector.tensor_reduce(
                z[:],
                acc[:].rearrange("p (j t) -> p t j", t=4),
                axis=X,
                op=add,
            )  # [128, 4, 7] -> [128, 4]
            logz = res.tile([128, PB], f32, tag="logz")
            nc.scalar.activation(logz[:], z[:], Ln, bias=zeros[:])
            v = res.tile([128, PB], f32, tag="v")
            vred = res.tile([128, 1], f32, tag="vred")
            nc.vector.tensor_mul(v[:], logz[:], w2)
            nc.vector.tensor_reduce(vred[:], v[:], axis=X, op=add)
            fin = psp.tile([128, 1024], f32, tag="mm")
            nc.tensor.matmul(fin[0:1, 0:1], ones, vred[:], start=True, stop=True)
            res_sb = res.tile([1, 1], f32, tag="res")
            nc.scalar.copy(res_sb[:], fin[0:1, 0:1])
            nc.sync.dma_start(out_d[:], res_sb[:])

    nc.compile()
    return nc


def _make_in_maps(features_i, features_ii, features_jj, i, ii, jj):
    f8 = ml_dtypes.float8_e4m3fn
    Fi = features_i.reshape(D, N).astype(np.float32)
    Fii = features_ii.reshape(D, N).astype(np.float32)
    Fjj = features_jj.reshape(D, N).astype(np.float32)
    lab = i.reshape(-1)
    ii_f = ii.reshape(-1)
    jj_f = jj.reshape(-1)

    cnt_ii = np.bincount(ii_f, minlength=L).astype(np.float32)
    cnt_jj = np.bincount(jj_f, minlength=L).astype(np.float32)
    wl = cnt_ii / (cnt_ii + cnt_jj + EPS)  # [L]

    perm_r = np.argsort(lab, kind="stable")
    lab_s = lab[perm_r]
    Fi_s = Fi[:, perm_r]
    perm_c = np.argsort(jj_f, kind="stable")
    jj_s = jj_f[perm_c]
    Fjj_s = (Fjj[:, perm_c] * FSCL).astype(f8)
    jstart = np.searchsorted(jj_s, np.arange(L), "left")
    jend = np.searchsorted(jj_s, np.arange(L), "right")

    dsum = (Fi * (Fii + Fjj)).sum(0) / TAU  # [N] diag1+diag2
    dsum_s = dsum[perm_r]
    w_s = wl[lab_s]

    rhs1 = np.zeros((128, N), np.float32)
    rhs1[0:D] = Fii
    rhs1[D : 2 * D] = Fii  # duplicate for the 64:128-partition stream
    rhs1 = (rhs1 * FSCL).astype(f8)

    in_maps = []
    wd_sums = []
    for c in range(NCORES):
        lhsP = np.zeros((128, 2 * 128), np.float32)
        rhs2 = np.zeros((128, 2 * 2 * SW), f8)
        small = np.zeros((128, 17), np.float32)
        small[:, 16] = 1.0
        for t in range(PB):
            g = PB * c + t
            q, quad = t // 2, t % 2
            rows = slice(g * 128, (g + 1) * 128)
            lhsP[quad * 64 : quad * 64 + 64, q * 128 : (q + 1) * 128] = Fi_s[:, rows]
            blk_lab = lab_s[rows]
            dl = np.unique(blk_lab)
            assert len(dl) <= 2, f"block {g} spans {len(dl)} labels"
            for s in range(2):
                if s < len(dl):
                    l = int(dl[s])
                    n_l = jend[l] - jstart[l]
                    assert n_l <= SW, f"label {l} has {n_l} cols > SW={SW}"
                    rhs2[
                        quad * 64 : quad * 64 + 64,
                        q * 2 * SW + s * SW : q * 2 * SW + s * SW + n_l,
                    ] = Fjj_s[:, jstart[l] : jend[l]]
                    small[:, t * 2 + s] = (blk_lab == l).astype(np.float32)
            # pad columns contribute exp(0)=1 to mask=1 rows: subtract here
            small[:, 8 + t] = EPS - (SW - cnt_jj[blk_lab])
            small[:, 12 + t] = 2.0 * w_s[rows] / N
        wd_sums.append(float((w_s[PB * c * 128 : PB * (c + 1) * 128]
                              * dsum_s[PB * c * 128 : PB * (c + 1) * 128]).sum()) / N)

        in_maps.append(
            {
                "lhsP": (lhsP * FSCL).astype(f8),
                "rhs1": rhs1,
                "rhs2": rhs2,
                "small": small,
            }
        )
    return in_maps, wd_sums


def kernel(features_i, features_ii, features_jj, i, ii, jj):
    global _compiled
    from concourse import bass_utils

    if _compiled is None:
        _compiled = _build()
    in_maps, wd_sums = _make_in_maps(
        features_i, features_ii, features_jj, i, ii, jj
    )
    results = bass_utils.run_bass_kernel_spmd(
        _compiled, in_maps, core_ids=list(range(NCORES))
    )
    total = np.float32(0.0)
    for c, r in enumerate(results.results):
        total += np.float32(r["out"].reshape(-1)[0]) - np.float32(wd_sums[c])
    return np.array(total, dtype=np.float32)


# revision 28
# speedup vs baseline: 1.0238x; 1.0238x over previous
"""Cross-image contrastive loss on 8 TRN2 NeuronCores.

Strategy (row-parallel over N=4096 pixels, 512 rows/core, rows sorted by label):
  - S1 (sum over exp(Fi.Fii/tau)): full [512, 4096] pass, fp8e4m3 matmuls
    (x8-scaled features; psum = 64*logits, folded into the exp scale).
  - S2 (label-masked sum): rows AND columns sorted by label; each 128-row
    block only needs the jj-columns matching its (at most 2) labels: a
    2x320-column window instead of 4096. The mask is applied by the DVE as
    a per-partition 0/1 multiply fused into the masked row-sum
    (tensor_scalar accum_out); zero-padded columns contribute exp(0)=1 and
    are subtracted exactly via a host-side per-row offset that rides in the
    accumulator alongside the +EPS term.
  - exp work is split between the Scalar engine (activation Exp, fused
    accum row-sum) and the Vector engine (Schraudolph exponential:
    y = A*psum + B computed into int32, bitcast to f32, summed; the
    bias constant is calibrated so the approximation is mean-zero, and it
    covers only 1/4 of each affected row's Z, keeping the loss error
    ~1e-4).
  - Histograms, per-pixel weights w, and the diagonal terms are O(N),
    computed on host; the device does all O(N^2) work.
  - Each core emits sum(2*w*logZ)/N; host subtracts sum(w*diag)/N and sums
    the 8 partials.
"""

import math
import sys

import numpy as np

sys.path.insert(0, "/opt/trn_rl_repo")

import ml_dtypes

TAU = 0.07
EPS = 1e-4
L = 19
D = 64
N = 4096
NCORES = 8
P = N // NCORES  # 512 rows per core
PB = P // 128  # 4 partition blocks per core
SW = 320  # S2 segment width (max label count in data ~238)
FSCL = 8.0  # fp8 feature scale; psum = FSCL^2 * logits
MSC = FSCL * FSCL
ESC = 1.0 / (MSC * TAU)  # exp scale applied to psum
# Schraudolph: exp(u) ~ bitcast_f32(int32(A*psum + B)); C calibrated mean-zero
SCH_C = 0.0562
SCH_A = (1 << 23) * ESC / math.log(2.0)
SCH_B = (1 << 23) * (127.0 - SCH_C) + 0.5

_compiled = None
_LDW_PATCHED = False


def _enable_ldw_opt():
    """Flip walrus --enable-ldw-opt for this process (dedups back-to-back
    LDWEIGHTS of the same stationary operand)."""
    global _LDW_PATCHED
    if _LDW_PATCHED:
        return
    from concourse import bass_utils

    orig = bass_utils.run_command

    def patched(cmd, *a, **kw):
        if isinstance(cmd, list):
            cmd = [
                "--enable-ldw-opt=true" if c == "--enable-ldw-opt=false" else c
                for c in cmd
            ]
        return orig(cmd, *a, **kw)

    bass_utils.run_command = patched
    _LDW_PATCHED = True


def _build():
    from concourse import bacc, mybir, tile

    f32 = mybir.dt.float32
    i32 = mybir.dt.int32
    bf16 = mybir.dt.bfloat16
    f8 = mybir.dt.float8e4
    Exp = mybir.ActivationFunctionType.Exp
    Ln = mybir.ActivationFunctionType.Ln
    X = mybir.AxisListType.X
    add = mybir.AluOpType.add
    mult = mybir.AluOpType.mult

    nc = bacc.Bacc("TRN2", target_bir_lowering=False, debug=False)

    lhsP_d = nc.dram_tensor("lhsP", (128, 2 * 128), f8, kind="ExternalInput")
    rhs1_d = nc.dram_tensor("rhs1", (128, N), f8, kind="ExternalInput")
    rhs2_d = nc.dram_tensor("rhs2", (128, 2 * 2 * SW), f8, kind="ExternalInput")
    small_d = nc.dram_tensor("small", (128, 17), f32, kind="ExternalInput")
    out_d = nc.dram_tensor("out", (1, 1), f32, kind="ExternalOutput")

    with tile.TileContext(nc) as tc:
        with (
            tc.tile_pool(name="res", bufs=1) as res,
            tc.tile_pool(name="scr", bufs=3) as scr,
            tc.tile_pool(name="yi", bufs=2) as yip,
            tc.tile_pool(name="ps", bufs=4, space="PSUM") as psp,
        ):
            lhsP_sb = res.tile([128, 2 * 128], f8, tag="lhsP")
            rhs1_sb = res.tile([128, N], f8, tag="rhs1")
            rhs2_sb = res.tile([128, 2 * 2 * SW], f8, tag="rhs2")
            small_sb = res.tile([128, 17], f32, tag="small")
            acc = res.tile([128, 28], f32, tag="acc")  # col = j*4 + t
            zeros = res.tile([128, 1], f32, tag="zeros")
            nc.vector.memset(zeros[:], 0.0)

            # DMA order: critical path (lhsP + first rhs1 pieces) first; the
            # leading 512-col pieces shrink time-to-first-matmul
            nc.sync.dma_start(rhs1_sb[:, 0:512], rhs1_d[:, 0:512])
            nc.scalar.dma_start(lhsP_sb[:], lhsP_d[:])
            for c0, c1 in ((512, 1024), (1024, 2048), (2048, 3072), (3072, 4096)):
                nc.sync.dma_start(rhs1_sb[:, c0:c1], rhs1_d[:, c0:c1])
            nc.scalar.dma_start(rhs2_sb[:], rhs2_d[:])
            nc.scalar.dma_start(small_sb[:], small_d[:])
            nc.sync.dma_start(acc[:, 24:28], small_d[:, 8:12])  # zoff as j=6

            nc.scalar.add_instruction(
                mybir.InstLoadActFuncSet(
                    name=nc.get_next_instruction_name(),
                    act_func_set_id=6,  # natural_log_exp_and_others
                    ins=[],
                    outs=[],
                )
            )

            mask = small_sb[:, 0:8]  # col t*2+s
            w2 = small_sb[:, 12:16]
            ones = small_sb[:, 16:17]

            def s1_unit(t, c, wsl, rsl, tp, dve):
                ps = psp.tile([128, 1024], f32, tag="mm")
                for m in range(2):
                    nc.tensor.matmul(
                        ps[:, m * 512 : (m + 1) * 512],
                        lhsP_sb[wsl[0] : wsl[1], wsl[2] : wsl[3]],
                        rhs1_sb[rsl[0] : rsl[1], c * 1024 + m * 512 : c * 1024 + (m + 1) * 512],
                        start=True,
                        stop=True,
                        tile_position=tp,
                    )
                col = c * 4 + t
                if dve:
                    yi = yip.tile([128, 1024], i32, tag="yi")
                    nc.vector.tensor_scalar(yi[:], ps[:], SCH_A, SCH_B, mult, add)
                    nc.vector.tensor_reduce(
                        acc[:, col : col + 1], yi[:].bitcast(f32), axis=X, op=add
                    )
                else:
                    dump = scr.tile([128, 1024], bf16, tag="dump")
                    nc.scalar.activation(
                        dump[:],
                        ps[:],
                        Exp,
                        bias=zeros[:],
                        scale=ESC,
                        accum_out=acc[:, col : col + 1],
                    )

            def s2_unit(t, q, wsl, rsl, tp):
                ps = psp.tile([128, 1024], f32, tag="mm")
                w0 = q * 2 * SW
                for c0, c1 in ((0, 512), (512, 2 * SW)):
                    nc.tensor.matmul(
                        ps[:, c0:c1],
                        lhsP_sb[wsl[0] : wsl[1], wsl[2] : wsl[3]],
                        rhs2_sb[rsl[0] : rsl[1], w0 + c0 : w0 + c1],
                        start=True,
                        stop=True,
                        tile_position=tp,
                    )
                dump = scr.tile([128, 2 * SW], bf16, tag="dump2")
                nc.scalar.activation(
                    dump[:], ps[:, 0 : 2 * SW], Exp, bias=zeros[:], scale=ESC
                )
                for s in range(2):
                    tmp = scr.tile([128, SW], bf16, tag="tmp2")
                    nc.vector.tensor_scalar(
                        tmp[:],
                        dump[:, s * SW : (s + 1) * SW],
                        mask[:, t * 2 + s : t * 2 + s + 1],
                        None,
                        mult,
                        add,
                        accum_out=acc[:, (4 + s) * 4 + t : (4 + s) * 4 + t + 1],
                    )

            for q in range(2):
                wA = (0, 64, q * 128, (q + 1) * 128)
                wB = (64, 128, q * 128, (q + 1) * 128)
                rA, rB = (0, 64), (64, 128)
                tA, tB = 2 * q, 2 * q + 1
                s1_unit(tA, 0, wA, rA, (0, 0), dve=False)
                s2_unit(tA, q, wA, rA, (0, 0))
                s1_unit(tA, 1, wA, rA, (0, 0), dve=True)
                s1_unit(tA, 2, wA, rA, (0, 0), dve=False)
                s1_unit(tA, 3, wA, rA, (0, 0), dve=False)
                s1_unit(tB, 0, wB, rB, (64, 0), dve=False)
                s2_unit(tB, q, wB, rB, (64, 0))
                s1_unit(tB, 1, wB, rB, (64, 0), dve=True)
                s1_unit(tB, 2, wB, rB, (64, 0), dve=(q == 0))
                s1_unit(tB, 3, wB, rB, (64, 0), dve=False)

            # ---- Z = sum_j acc[j, t] (incl. zoff), logZ, partial ----
            z = res.tile([128, PB], f32, tag="z")
            nc.vector.tensor_reduce(
                z[:],
                acc[:].rearrange("p (j t) -> p t j", t=4),
                axis=X,
                op=add,
            )  # [128, 4, 7] -> [128, 4]
            logz = res.tile([128, PB], f32, tag="logz")
            nc.scalar.activation(logz[:], z[:], Ln, bias=zeros[:])
            v = res.tile([128, PB], f32, tag="v")
            vred = res.tile([128, 1], f32, tag="vred")
            nc.vector.tensor_mul(v[:], logz[:], w2)
            nc.vector.tensor_reduce(vred[:], v[:], axis=X, op=add)
            fin = psp.tile([128, 1024], f32, tag="mm")
            nc.tensor.matmul(fin[0:1, 0:1], ones, vred[:], start=True, stop=True)
            res_sb = res.tile([1, 1], f32, tag="res")
            nc.scalar.copy(res_sb[:], fin[0:1, 0:1])
            nc.sync.dma_start(out_d[:], res_sb[:])

    nc.compile()
    return nc


def _make_in_maps(features_i, features_ii, features_jj, i, ii, jj):
    f8 = ml_dtypes.float8_e4m3fn
    Fi = features_i.reshape(D, N).astype(np.float32)
    Fii = features_ii.reshape(D, N).astype(np.float32)
    Fjj = features_jj.reshape(D, N).astype(np.float32)
    lab = i.reshape(-1)
    ii_f = ii.reshape(-1)
    jj_f = jj.reshape(-1)

    cnt_ii = np.bincount(ii_f, minlength=L).astype(np.float32)
    cnt_jj = np.bincount(jj_f, minlength=L).astype(np.float32)
    wl = cnt_ii / (cnt_ii + cnt_jj + EPS)  # [L]

    perm_r = np.argsort(lab, kind="stable")
    lab_s = lab[perm_r]
    Fi_s = Fi[:, perm_r]
    perm_c = np.argsort(jj_f, kind="stable")
    jj_s = jj_f[perm_c]
    Fjj_s = (Fjj[:, perm_c] * FSCL).astype(f8)
    jstart = np.searchsorted(jj_s, np.arange(L), "left")
    jend = np.searchsorted(jj_s, np.arange(L), "right")

    dsum = (Fi * (Fii + Fjj)).sum(0) / TAU  # [N] diag1+diag2
    dsum_s = dsum[perm_r]
    w_s = wl[lab_s]

    rhs1 = np.zeros((128, N), np.float32)
    rhs1[0:D] = Fii
    rhs1[D : 2 * D] = Fii  # duplicate for the 64:128-partition stream
    rhs1 = (rhs1 * FSCL).astype(f8)

    in_maps = []
    wd_sums = []
    for c in range(NCORES):
        lhsP = np.zeros((128, 2 * 128), np.float32)
        rhs2 = np.zeros((128, 2 * 2 * SW), f8)
        small = np.zeros((128, 17), np.float32)
        small[:, 16] = 1.0
        for t in range(PB):
            g = PB * c + t
            q, quad = t // 2, t % 2
            rows = slice(g * 128, (g + 1) * 128)
            lhsP[quad * 64 : quad * 64 + 64, q * 128 : (q + 1) * 128] = Fi_s[:, rows]
            blk_lab = lab_s[rows]
            dl = np.unique(blk_lab)
            assert len(dl) <= 2, f"block {g} spans {len(dl)} labels"
            for s in range(2):
                if s < len(dl):
                    l = int(dl[s])
                    n_l = jend[l] - jstart[l]
                    assert n_l <= SW, f"label {l} has {n_l} cols > SW={SW}"
                    rhs2[
                        quad * 64 : quad * 64 + 64,
                        q * 2 * SW + s * SW : q * 2 * SW + s * SW + n_l,
                    ] = Fjj_s[:, jstart[l] : jend[l]]
                    small[:, t * 2 + s] = (blk_lab == l).astype(np.float32)
            # pad columns contribute exp(0)=1 to mask=1 rows: subtract here
            small[:, 8 + t] = EPS - (SW - cnt_jj[blk_lab])
            small[:, 12 + t] = 2.0 * w_s[rows] / N
        wd_sums.append(float((w_s[PB * c * 128 : PB * (c + 1) * 128]
                              * dsum_s[PB * c * 128 : PB * (c + 1) * 128]).sum()) / N)

        in_maps.append(
            {
                "lhsP": (lhsP * FSCL).astype(f8),
                "rhs1": rhs1,
                "rhs2": rhs2,
                "small": small,
            }
        )
    return in_maps, wd_sums


def kernel(features_i, features_ii, features_jj, i, ii, jj):
    global _compiled
    from concourse import bass_utils

    if _compiled is None:
        _compiled = _build()
    in_maps, wd_sums = _make_in_maps(
        features_i, features_ii, features_jj, i, ii, jj
    )
    results = bass_utils.run_bass_kernel_spmd(
        _compiled, in_maps, core_ids=list(range(NCORES))
    )
    total = np.float32(0.0)
    for c, r in enumerate(results.results):
        total += np.float32(r["out"].reshape(-1)[0]) - np.float32(wd_sums[c])
    return np.array(total, dtype=np.float32)
